# revision 62
# baseline (speedup 1.0000x reference)
"""DSConv (deformable "snake" conv block) Trainium2 Bass kernel.

Reference math (per batch b):
  off   = conv3x3(f) + off_b -> BN(eval) -> tanh ; x_off = channels 9:18
  cum   = cumulative offsets along k from center (matrix `tri`)
  X     = h + x_off_cum (sampling x-coord; y = w + k - 4 is always integer)
  samp[b,c,w,h,k] = (0<=y<=126 and 0<=X<127) ? lerp_x(f[b,c,y,:], X) : 0
  out   = snake conv: out[b,co,w,h] = sum_{ci,k} conv_w[co,ci,0,k] samp[b,ci,w,h,k]
  out   = GroupNorm(32 groups over (4co,W,H)) -> ReLU
Sharding: 8 cores = 2 batch x 4 W-quarters (32 output rows each).

Per-core pipeline (fp16 matmul operands, fp32 PSUM):
  offset  conv3x3 as [96-part] matmul chunks; dh recombined by Act copy +
          2 DVE shifted adds; tanh(+bvec) on Act; cum = [18,9] matmul that
          also folds the invalid-y BIG bias (ybt rows) in its contraction.
  cliffs  the bilinear clip makes the reference discontinuous at X=0/127
          (only reachable at h in {0,1,2} u {125,126,127} since |cum|<3);
          those 10 fcv columns are recomputed with hi/lo-f16 conv and fp32
          cum so the mask decision matches the fp32 reference to ~1e-5.
  braw16  [9,(k-major wl,h)] f16 = -cum (+BIG where invalid), folded into
          rhs3 row 2 by ONE flattening DMA (rows 0/1 = const 1 and -h).
  D(wl)   [x,(k,h)] = x - h - cum via one contraction-3 matmul (PE).
  negA    = min(|D|,1)-1 = -tent(x-X): DVE copy + sign-bit clear for one
          384-chunk, Act Abs for the rest, one 4x DVE (min,sub).
  step2   po[co,h] += q[wl+k][:,k,:]^T @ negA[:,k,:]; lhsT=Q so output
          lands [co,h] pre-transposed; DVE drains to outC (GPSIMD cannot
          touch PSUM on real HW).
  GN      bn_stats -> group matmuls -> AllGather (15us vs AllReduce 28us
          in the cost model) -> local reduce -> relu(S*x+B) with the tent
          sign folded into S.
"""

import numpy as np
from contextlib import ExitStack

import concourse.bass as bass
import concourse.bacc as bacc
import concourse.tile as tile
import concourse.mybir as mybir
from concourse import bass_utils

F16 = mybir.dt.float16
F32 = mybir.dt.float32
U8 = mybir.dt.uint8
ALU = mybir.AluOpType
ACTF = mybir.ActivationFunctionType

K = 9
CENTER = 4
P = 128
W = 128
H = 128
B = 2
NY = 40            # sampling row window per core: y in [w0-4, w0+36)
WC = 32            # output w-rows per core
EPS = 1e-5
EXTEND = 1.0
BIG = 30000.0
NCORES = 8
NKH = K * P        # 1152
SPL = 864          # DVE share of the negA abs/min pass


def _tri_base():
    """tri[k, kp] = coeff of x_off[k] in cumulative offset new[kp]."""
    t = np.zeros((K, K), np.float32)
    t[0, 0] = 1.0
    t[K - 1, K - 1] = 1.0
    for i in range(1, CENTER):
        for j in range(CENTER + 1, CENTER + i + 1):
            t[j, CENTER + i] = 1.0
        for j in range(CENTER - i, CENTER):
            t[j, CENTER - i] = 1.0
    return t


def build_nc():
    nc = bacc.Bacc("TRN2", target_bir_lowering=False, debug=False,
                   num_devices=NCORES)

    fs_d = nc.dram_tensor("fs", [P, NY, P], F16, kind="ExternalInput")
    fcvh_d = nc.dram_tensor("fcvh", [P, 34, 130], F16, kind="ExternalInput")
    wt_d = nc.dram_tensor("wt", [P, NKH], F16, kind="ExternalInput")
    owh_d = nc.dram_tensor("owh", [P, 3, 96], F16, kind="ExternalInput")
    bvec_d = nc.dram_tensor("bvec", [K, 1], F32, kind="ExternalInput")
    triext_d = nc.dram_tensor("triext", [2 * K, K], F16, kind="ExternalInput")
    ybt_d = nc.dram_tensor("ybt", [K, WC * P], F16, kind="ExternalInput")
    hlo32_d = nc.dram_tensor("hlo32", [K, 3], F32, kind="ExternalInput")
    hhi32_d = nc.dram_tensor("hhi32", [K, 3], F32, kind="ExternalInput")
    big32_d = nc.dram_tensor("big32", [K, 1], F32, kind="ExternalInput")
    owl_d = nc.dram_tensor("owl", [P, 3, 96], F16, kind="ExternalInput")
    triext32_d = nc.dram_tensor("triext32", [2 * K, K], F32,
                                kind="ExternalInput")
    fcvbh_d = nc.dram_tensor("fcvbh", [P, 34, 10], F16, kind="ExternalInput")
    fcvbl_d = nc.dram_tensor("fcvbl", [P, 34, 10], F16, kind="ExternalInput")
    ybtb_d = nc.dram_tensor("ybtb", [K, WC * 6], F32, kind="ExternalInput")
    lhsT3_d = nc.dram_tensor("lhsT3", [3, P], F16, kind="ExternalInput")
    rhsC_d = nc.dram_tensor("rhsC", [2, WC * NKH], F16, kind="ExternalInput")
    gmat_d = nc.dram_tensor("gmat", [P, 32], F32, kind="ExternalInput")
    gexp_d = nc.dram_tensor("gexp", [32, P], F32, kind="ExternalInput")
    gam_d = nc.dram_tensor("gam", [P, 1], F32, kind="ExternalInput")
    bet_d = nc.dram_tensor("bet", [P, 1], F32, kind="ExternalInput")
    out_d = nc.dram_tensor("out", [P, WC, P], F32, kind="ExternalOutput")

    cr_in = nc.dram_tensor("cr_in", [32, 2], F32, kind="Internal")
    cr_out = nc.dram_tensor("cr_out", [4 * 32, 2], F32, kind="Internal")

    with tile.TileContext(nc) as tc, ExitStack() as ctx:
        const = ctx.enter_context(tc.tile_pool(name="const", bufs=1))
        mid = ctx.enter_context(tc.tile_pool(name="mid", bufs=1))

        def load(name, dram, shape, dtype):
            t = const.tile(shape, dtype, name=name + "_sb")
            nc.sync.dma_start(out=t, in_=dram.ap())
            return t

        owh = load("owh", owh_d, [P, 3, 96], F16)
        bvec = load("bvec", bvec_d, [K, 1], F32)
        triext = load("triext", triext_d, [2 * K, K], F16)
        fcvh = const.tile([P, 34, 130], F16, name="fcvh_sb")
        for lo, hi in ((0, 6), (6, 14), (14, 24), (24, 34)):
            nc.sync.dma_start(out=fcvh[:, lo:hi, :],
                              in_=fcvh_d.ap()[:, lo:hi, :])
        xoff18 = const.tile([2 * K, WC, P], F16, name="xoff18_sb")
        nc.sync.dma_start(
            out=xoff18.rearrange("p w h -> p (w h)")[K:2 * K, :],
            in_=ybt_d.ap())
        wt = load("wt", wt_d, [P, NKH], F16)
        fs = const.tile([P, NY, P], F16, name="fs_sb")
        for lo, hi in ((0, 10), (10, 24), (24, NY)):
            nc.sync.dma_start(out=fs[:, lo:hi, :],
                              in_=fs_d.ap()[:, lo:hi, :])
        hlo32 = load("hlo32", hlo32_d, [K, 3], F32)
        hhi32 = load("hhi32", hhi32_d, [K, 3], F32)
        big32 = load("big32", big32_d, [K, 1], F32)
        owl = load("owl", owl_d, [P, 3, 96], F16)
        triext32 = load("triext32", triext32_d, [2 * K, K], F32)
        fcvbh4 = load("fcvbh", fcvbh_d, [P, 34, 10], F16)
        fcvbl4 = load("fcvbl", fcvbl_d, [P, 34, 10], F16)
        fcvbh = fcvbh4.rearrange("p w (s c) -> p w s c", s=2)
        fcvbl = fcvbl4.rearrange("p w (s c) -> p w s c", s=2)
        xoffb32 = const.tile([2 * K, WC, 2, 3], F32, name="xoffb32_sb")
        nc.sync.dma_start(
            out=xoffb32.rearrange("p w s c -> p (w s c)")[K:2 * K, :],
            in_=ybtb_d.ap())
        lhsT3 = load("lhsT3", lhsT3_d, [3, P], F16)
        rhs3 = const.tile([3, WC * NKH], F16, name="rhs3_sb")
        nc.sync.dma_start(out=rhs3[0:2, :], in_=rhsC_d.ap())
        gmat = load("gmat", gmat_d, [P, 32], F32)
        gexp = load("gexp", gexp_d, [32, P], F32)
        gam = load("gam", gam_d, [P, 1], F32)
        bet = load("bet", bet_d, [P, 1], F32)

        braw16 = mid.tile([K, WC, P], F16)
        bb32 = mid.tile([K, WC, 2, 3], F32)
        outC = mid.tile([P, WC, P], F16)
        stats = mid.tile([P, 8, 6], F32)

        qtiles = {}
        qpool = ctx.enter_context(tc.tile_pool(name="qpool", bufs=13))
        napool = ctx.enter_context(tc.tile_pool(name="napool", bufs=5))
        fpool = ctx.enter_context(tc.tile_pool(name="fpool", bufs=4))
        psQ = ctx.enter_context(tc.tile_pool(name="psQ", bufs=5,
                                             space="PSUM"))
        psM = ctx.enter_context(tc.tile_pool(name="psM", bufs=1,
                                             space="PSUM"))

        nq = [0]

        def q_row(yl, engs="s"):
            q = qpool.tile([P, K, P], F16, tag="q", name="q_sb")
            qf = q.rearrange("p k c -> p (k c)")
            for j in range(3):
                psq = psQ.tile([P, 384], F32, tag="q", name="psq")
                nc.tensor.matmul(psq, lhsT=fs[:, yl, :],
                                 rhs=wt[:, j * 384:(j + 1) * 384],
                                 start=True, stop=True)
                dst = qf[:, j * 384:(j + 1) * 384]
                if engs[j % len(engs)] == "v":
                    nc.vector.tensor_copy(out=dst, in_=psq)
                else:
                    nc.scalar.copy(out=dst, in_=psq)
            nq[0] += 1
            qtiles[yl] = q

        # ---- offset branch (+ first 9 q rows interleaved) ----
        with tc.tile_pool(name="x9p", bufs=4) as x9p, \
             tc.tile_pool(name="psC", bufs=2, space="PSUM") as psC:
            wchunks = [(c3, min(3, WC - c3)) for c3 in range(0, WC, 3)]
            for ci_, (c3, nw) in enumerate(wchunks):
                pss = psC.tile([96, 3, 130], F32, tag="cv", name="pss")
                for dw in range(3):
                    nc.tensor.matmul(pss[:, :nw, :], lhsT=owh[:, dw, :],
                                     rhs=fcvh[:, c3 + dw: c3 + dw + nw, :],
                                     start=(dw == 0), stop=(dw == 2))
                g0 = x9p.tile([K, 3, P], F32, tag="g0", name="g0")
                g1 = x9p.tile([K, 3, P], F32, tag="g1", name="g1")
                x9t = x9p.tile([K, 3, P], F32, tag="x9", name="x9t")
                # the bir verifier allows only one PSUM input per op
                nc.scalar.copy(out=g0[:, :nw, :], in_=pss[0:K, :nw, 0:128])
                eng1 = nc.vector
                eng2 = nc.vector
                eng1.tensor_tensor(out=g1[:, :nw, :],
                                   in0=pss[32:32 + K, :nw, 1:129],
                                   in1=g0[:, :nw, :], op=ALU.add)
                eng2.tensor_tensor(out=x9t[:, :nw, :],
                                   in0=pss[64:64 + K, :nw, 2:130],
                                   in1=g1[:, :nw, :], op=ALU.add)
                nc.scalar.activation(out=xoff18[0:K, c3:c3 + nw, :],
                                     in_=x9t[:, :nw, :], func=ACTF.Tanh,
                                     bias=bvec, scale=1.0)
                pcum = psM.tile([K, 3, P], F32, tag="cm", name="pcum")
                nc.tensor.matmul(pcum[:, :nw, :], lhsT=triext,
                                 rhs=xoff18[:, c3:c3 + nw, :],
                                 start=True, stop=True)
                if ci_ % 2 == 0:
                    nc.vector.tensor_copy(out=braw16[:, c3:c3 + nw, :],
                                          in_=pcum[:, :nw, :])
                else:
                    nc.scalar.copy(out=braw16[:, c3:c3 + nw, :],
                                   in_=pcum[:, :nw, :])
                if ci_ < K:
                    q_row(ci_, engs=("vvs", "svv", "vsv")[ci_ % 3])

            # ---- precise boundary path ----
            # The bilinear clip makes the reference DISCONTINUOUS at X=0 and
            # X=127, which (|cum|<3) can only hit output columns h in
            # {0,1,2} u {125,126,127}. Those samples' cum must match the
            # fp32 reference to ~1e-5 or a sample can land on the wrong side
            # of the cliff (O(1) error). Recompute just the 10 affected fcv
            # columns with hi/lo f16 conv + fp32 cum, decide the masks there,
            # and overwrite braw16's 6 boundary columns.
            pssb = psC.tile([96, WC, 2, 5], F32, tag="cv", name="pssb")
            for dw in range(3):
                rh = fcvbh[:, dw:dw + WC, :, :]
                rl = fcvbl[:, dw:dw + WC, :, :]
                nc.tensor.matmul(pssb, lhsT=owh[:, dw, :], rhs=rh,
                                 start=(dw == 0), stop=False)
                nc.tensor.matmul(pssb, lhsT=owl[:, dw, :], rhs=rh,
                                 start=False, stop=False)
                nc.tensor.matmul(pssb, lhsT=owh[:, dw, :], rhs=rl,
                                 start=False, stop=(dw == 2))
            g0b = x9p.tile([K, WC, 2, 3], F32, tag="g0", name="g0b")
            g1b = x9p.tile([K, WC, 2, 3], F32, tag="g1", name="g1b")
            x9b = x9p.tile([K, WC, 2, 3], F32, tag="x9", name="x9b")
            nc.scalar.copy(out=g0b, in_=pssb[0:K, :, :, 0:3])
            nc.vector.tensor_tensor(out=g1b, in0=pssb[32:32 + K, :, :, 1:4],
                                    in1=g0b, op=ALU.add)
            nc.vector.tensor_tensor(out=x9b, in0=pssb[64:64 + K, :, :, 2:5],
                                    in1=g1b, op=ALU.add)
            nc.scalar.activation(out=xoffb32[0:K, :, :, :], in_=x9b,
                                 func=ACTF.Tanh, bias=bvec, scale=1.0)
            pcumb = psM.tile([K, WC, 2, 3], F32, tag="cm", name="pcumb")
            nc.tensor.matmul(pcumb, lhsT=triext32,
                             rhs=xoffb32.rearrange("p w s c -> p (w s c)"),
                             start=True, stop=True)
            bb32 = mid.tile([K, WC, 2, 3], F32)
            nc.vector.tensor_copy(out=bb32, in_=pcumb)
            m1 = mid.tile([K, WC, 3], U8)
            m2 = mid.tile([K, WC, 3], U8)
            nc.vector.tensor_tensor(
                out=m1, in0=bb32[:, :, 0, :],
                in1=hlo32.unsqueeze(1).to_broadcast([K, WC, 3]), op=ALU.is_gt)
            nc.vector.copy_predicated(
                out=bb32[:, :, 0, :], mask=m1,
                data=big32.to_broadcast([K, WC, 3]))
            nc.vector.tensor_tensor(
                out=m2, in0=bb32[:, :, 1, :],
                in1=hhi32.unsqueeze(1).to_broadcast([K, WC, 3]), op=ALU.is_le)
            nc.vector.copy_predicated(
                out=bb32[:, :, 1, :], mask=m2,
                data=big32.to_broadcast([K, WC, 3]))
            nc.vector.tensor_copy(out=braw16[:, :, 0:3], in_=bb32[:, :, 0, :])
            nc.vector.tensor_copy(out=braw16[:, :, 125:128],
                                  in_=bb32[:, :, 1, :])

        # fold braw16 into rhs3 row 2; row-2 layout is (k, wl, h) which is
        # exactly braw16's storage order -> one flattening DMA
        nc.sync.dma_start(
            out=rhs3[2:3, :],
            in_=braw16.rearrange("p w h -> p (w h)"))
        rhs3v = rhs3.rearrange("p (k w h) -> p k w h", k=K, h=P)

        # ---- main loop ----
        with tc.tile_pool(name="psD", bufs=2, space="PSUM") as psD:

            def emit_iter(wl):
                """Software-pipelined: build D/negA for wl+1, step2 for wl."""
                nxt = wl + 1
                na = None
                if nxt <= WC - 1:
                    na = napool.tile([P, K, P], F16, tag="na", name="na")
                    naf = na.rearrange("p k h -> p (k h)")
                    for j in range(3):
                        pd = psD.tile([P, 384], F32, tag="d", name="psd")
                        nc.tensor.matmul(
                            pd, lhsT=lhsT3,
                            rhs=rhs3v[:, 3 * j:3 * j + 3, nxt, :],
                            start=True, stop=True)
                        dst = naf[:, j * 384:(j + 1) * 384]
                        if j == 0:
                            nc.vector.tensor_copy(out=dst, in_=pd)
                        else:
                            nc.scalar.activation(out=dst, in_=pd,
                                                 func=ACTF.Abs)
                if 0 <= wl and wl + K < NY:
                    q_row(wl + K, engs="ssv")
                if na is not None:
                    # |d| for the DVE-copied part: clear the f16 sign bit
                    nau = naf[:, 0:384].bitcast(mybir.dt.uint16)
                    nc.vector.tensor_scalar(out=nau, in0=nau, scalar1=0x7FFF,
                                            scalar2=None, op0=ALU.bitwise_and)
                    nc.vector.tensor_scalar(out=naf, in0=naf, scalar1=1.0,
                                            scalar2=1.0, op0=ALU.min,
                                            op1=ALU.subtract)
                if wl < 0:
                    return na
                po = psM.tile([P, P], F32, tag="cm", name="po")
                na_c = nas.pop(wl)
                for k in range(K):
                    nc.tensor.matmul(po, lhsT=qtiles[wl + k][:, k, :],
                                     rhs=na_c[:, k, :],
                                     start=(k == 0), stop=(k == K - 1))
                nc.vector.tensor_copy(out=outC[:, wl, :], in_=po)
                if wl % 4 == 0 and wl > 0:
                    nc.vector.bn_stats(
                        out=stats[:, wl // 4 - 1, :],
                        in_=outC[:, wl - 4:wl, :].rearrange(
                            "p a b -> p (a b)"))
                del qtiles[wl]
                return na

            nas = {}
            nas[0] = emit_iter(-1)
            for wl in range(WC):
                na = emit_iter(wl)
                if na is not None:
                    nas[wl + 1] = na

            # ---- GroupNorm ----
            nc.vector.bn_stats(
                out=stats[:, 7, :],
                in_=outC[:, WC - 4:WC, :].rearrange("p a b -> p (a b)"))
            mv = mid.tile([P, 2], F32)
            nc.vector.bn_aggr(out=mv, in_=stats)
            st2 = mid.tile([P, 2], F32)
            nc.gpsimd.tensor_copy(out=st2[:, 0:1], in_=mv[:, 0:1])
            sq = mid.tile([P, 1], F32)
            nc.gpsimd.tensor_tensor(out=sq, in0=mv[:, 0:1], in1=mv[:, 0:1],
                                    op=ALU.mult)
            nc.gpsimd.tensor_tensor(out=st2[:, 1:2], in0=mv[:, 1:2], in1=sq,
                                    op=ALU.add)
            pg = psM.tile([32, 2], F32, tag="cm", name="ps_g")
            nc.tensor.matmul(pg, lhsT=gmat, rhs=st2, start=True, stop=True)
            g2 = mid.tile([32, 2], F32)
            nc.vector.tensor_copy(out=g2, in_=pg)
            nc.sync.dma_start(out=cr_in.ap(), in_=g2)
            nc.gpsimd.collective_compute(
                kind="AllGather", op=ALU.bypass,
                replica_groups=[[0, 1, 2, 3], [4, 5, 6, 7]],
                ins=[cr_in.ap()], outs=[cr_out.ap()])
            gg = mid.tile([32, 4, 2], F32)
            nc.sync.dma_start(
                out=gg, in_=cr_out.ap().rearrange("(r c) v -> c r v", r=4))
            g2s = mid.tile([32, 2], F32)
            # sum over the 4 cores: view [32, 2, 4] (v outer, r inner)
            nc.vector.tensor_reduce(
                out=g2s, in_=gg.rearrange("c r v -> c v r"),
                axis=mybir.AxisListType.X, op=ALU.add)
            m2 = mid.tile([32, 1], F32)
            nc.vector.tensor_tensor(out=m2, in0=g2s[:, 0:1], in1=g2s[:, 0:1],
                                    op=ALU.mult)
            vg = mid.tile([32, 1], F32)
            nc.vector.tensor_tensor(out=vg, in0=g2s[:, 1:2], in1=m2,
                                    op=ALU.subtract)
            nc.vector.tensor_scalar(out=vg, in0=vg, scalar1=EPS,
                                    scalar2=None, op0=ALU.add)
            nc.scalar.sqrt(out=vg, in_=vg)
            nc.vector.reciprocal(out=vg, in_=vg)
            g3 = mid.tile([32, 2], F32)
            nc.vector.tensor_copy(out=g3[:, 0:1], in_=g2s[:, 0:1])
            nc.vector.tensor_copy(out=g3[:, 1:2], in_=vg)
            pe2 = psM.tile([P, 2], F32, tag="cm", name="ps_e2")
            nc.tensor.matmul(pe2, lhsT=gexp, rhs=g3, start=True, stop=True)
            ec = mid.tile([P, 2], F32)
            nc.vector.tensor_copy(out=ec, in_=pe2)
            t1 = mid.tile([P, 1], F32)
            nc.vector.tensor_tensor(out=t1, in0=ec[:, 1:2], in1=gam,
                                    op=ALU.mult)
            Sv = mid.tile([P, 1], F32)
            nc.vector.tensor_scalar(out=Sv, in0=t1, scalar1=-1.0, scalar2=None,
                                    op0=ALU.mult)
            t2 = mid.tile([P, 1], F32)
            nc.vector.tensor_tensor(out=t2, in0=ec[:, 0:1], in1=t1,
                                    op=ALU.mult)
            Bv = mid.tile([P, 1], F32)
            nc.vector.tensor_tensor(out=Bv, in0=t2, in1=bet, op=ALU.add)
            blocks = ((0, 4), (4, 12), (12, 22), (22, WC))
            for b0, b1 in blocks:
                fin = fpool.tile([P, b1 - b0, P], F32, tag="f", name="fin")
                nc.scalar.activation(out=fin, in_=outC[:, b0:b1, :],
                                     func=ACTF.Relu, bias=Bv, scale=Sv)
                nc.sync.dma_start(out=out_d.ap()[:, b0:b1, :], in_=fin)

    nc.compile()
    return nc


_TRI = _tri_base()


def prep_shared(off_w, off_b, bn_gamma, bn_beta, bn_mean, bn_var, conv_w,
                gn_gamma, gn_beta):
    s36 = (np.asarray(bn_gamma, np.float32)
           / np.sqrt(np.asarray(bn_var, np.float32) + EPS))
    s = s36[K:2 * K]
    bvec = ((np.asarray(off_b, np.float32)[K:2 * K]
             - np.asarray(bn_mean, np.float32)[K:2 * K]) * s
            + np.asarray(bn_beta, np.float32)[K:2 * K]
            ).reshape(K, 1).astype(np.float32)

    owf = np.asarray(off_w, np.float32)[K:2 * K]          # [k, ci, dw, dh]
    oww = np.zeros((P, 3, 96), np.float32)                # [ci, dw, (dh-group, k)]
    for dw in range(3):
        for dh in range(3):
            oww[:, dw, dh * 32: dh * 32 + K] = (owf[:, :, dw, dh] * s[:, None]).T
    owh = oww.astype(np.float16)
    owl = (oww - owh.astype(np.float32)).astype(np.float16)

    wtf = np.asarray(conv_w, np.float32)[:, :, 0, :]      # [co, ci, k]
    wt = np.ascontiguousarray(
        np.transpose(wtf, (1, 2, 0)).reshape(P, K * P)).astype(np.float16)

    hx = np.arange(P, dtype=np.float32)
    mask7 = np.full((P, 1), 0x7FFF, np.uint16)
    triext32 = np.concatenate([-EXTEND * _TRI,
                               np.eye(K, dtype=np.float32)], axis=0)
    triext = triext32.astype(np.float16)
    lhsT3 = np.stack([hx, np.ones(P, np.float32),
                      np.ones(P, np.float32)]).astype(np.float16)
    rhsC = np.stack([np.ones(WC * K * P, np.float32),
                     np.tile(-hx, WC * K)]).astype(np.float16)
    return dict(
        wt=wt, owh=owh, owl=owl, bvec=bvec, triext=triext,
        triext32=triext32.astype(np.float32), lhsT3=lhsT3, rhsC=rhsC,
        hlo32=np.tile(np.arange(3, dtype=np.float32), (K, 1)),
        hhi32=np.tile(np.arange(3, dtype=np.float32) - 2.0, (K, 1)),
        big32=np.full((K, 1), BIG, np.float32),
        gam=np.asarray(gn_gamma, np.float32).reshape(P, 1),
        bet=np.asarray(gn_beta, np.float32).reshape(P, 1),
        gmat=np.array([[0.0625 if co // 4 == g else 0.0 for g in range(32)]
                       for co in range(P)], np.float32),
        gexp=np.array([[1.0 if co // 4 == g else 0.0 for co in range(P)]
                       for g in range(32)], np.float32),
    )


def prep_core(f, b, w0):
    fb = np.asarray(f, np.float32)[b]
    fs = np.zeros((P, NY, P), np.float16)
    lo = max(0, w0 - 4)
    hi = min(W, w0 + 36)
    fs[:, lo - (w0 - 4): hi - (w0 - 4), :] = fb[:, lo:hi, :].astype(np.float16)
    fpad = np.pad(fb, ((0, 0), (1, 1), (1, 1)))
    fcvh = np.ascontiguousarray(fpad[:, w0:w0 + 34, :]).astype(np.float16)
    b10 = [0, 1, 2, 3, 4, 125, 126, 127, 128, 129]
    fcvb = np.ascontiguousarray(fpad[:, w0:w0 + 34, :][:, :, b10])
    fcvbh = fcvb.astype(np.float16)
    fcvbl = (fcvb - fcvbh.astype(np.float32)).astype(np.float16)
    ybt = np.zeros((K, WC, P), np.float16)
    ybtb = np.zeros((K, WC, 6), np.float32)
    for wl in range(WC):
        for k in range(K):
            y = w0 + wl + k - 4
            if not (0 <= y <= 126):
                ybt[k, wl, :] = BIG
                ybtb[k, wl, :] = BIG
    return dict(fs=fs, fcvh=fcvh, ybt=ybt.reshape(K, WC * P),
                fcvbh=fcvbh, fcvbl=fcvbl, ybtb=ybtb.reshape(K, WC * 6))


_NC_CACHE = {}


def get_nc():
    if "nc" not in _NC_CACHE:
        _NC_CACHE["nc"] = build_nc()
    return _NC_CACHE["nc"]


def make_in_maps(f, off_w, off_b, bn_gamma, bn_beta, bn_mean, bn_var,
                 conv_w, conv_b, gn_gamma, gn_beta):
    consts = prep_shared(off_w, off_b, bn_gamma, bn_beta, bn_mean, bn_var,
                         conv_w, gn_gamma, gn_beta)
    in_maps = []
    for c in range(NCORES):
        b, q = c // 4, c % 4
        m = dict(consts)
        m.update(prep_core(f, b, q * WC))
        in_maps.append(m)
    return in_maps


def assemble(results):
    out = np.zeros((B, P, W, H), np.float32)
    for c in range(NCORES):
        b, q = c // 4, c % 4
        out[b, :, q * WC:(q + 1) * WC, :] = results[c]["out"]
    return out


def kernel(f, off_w, off_b, bn_gamma, bn_beta, bn_mean, bn_var,
           conv_w, conv_b, gn_gamma, gn_beta, **run_kwargs):
    nc = get_nc()
    in_maps = make_in_maps(f, off_w, off_b, bn_gamma, bn_beta, bn_mean,
                           bn_var, conv_w, conv_b, gn_gamma, gn_beta)
    last_exc = None
    for _attempt in range(3):
        try:
            res = bass_utils.run_bass_kernel_spmd(
                nc, in_maps, core_ids=list(range(NCORES)), **run_kwargs)
            break
        except Exception as e:  # transient tunnel/device hiccups
            last_exc = e
    else:
        raise last_exc
    out = assemble(res.results)
    kernel.last_result = res
    return out


# revision 63
# speedup vs baseline: 1.0225x; 1.0225x over previous
"""DSConv (deformable "snake" conv block) Trainium2 Bass kernel.

Reference math (per batch b):
  off   = conv3x3(f) + off_b -> BN(eval) -> tanh ; x_off = channels 9:18
  cum   = cumulative offsets along k from center (matrix `tri`)
  X     = h + x_off_cum (sampling x-coord; y = w + k - 4 is always integer)
  samp[b,c,w,h,k] = (0<=y<=126 and 0<=X<127) ? lerp_x(f[b,c,y,:], X) : 0
  out   = snake conv: out[b,co,w,h] = sum_{ci,k} conv_w[co,ci,0,k] samp[b,ci,w,h,k]
  out   = GroupNorm(32 groups over (4co,W,H)) -> ReLU
Sharding: 8 cores = 2 batch x 4 W-quarters (32 output rows each).

Per-core pipeline (fp16 matmul operands, fp32 PSUM):
  offset  conv3x3 as [96-part] matmul chunks; dh recombined by Act copy +
          2 DVE shifted adds; tanh(+bvec) on Act; cum = [18,9] matmul that
          also folds the invalid-y BIG bias (ybt rows) in its contraction.
  cliffs  the bilinear clip makes the reference discontinuous at X=0/127
          (only reachable at h in {0,1,2} u {125,126,127} since |cum|<3);
          those 10 fcv columns are recomputed with hi/lo-f16 conv and fp32
          cum so the mask decision matches the fp32 reference to ~1e-5.
  braw16  [9,(k-major wl,h)] f16 = -cum (+BIG where invalid), folded into
          rhs3 row 2 by ONE flattening DMA (rows 0/1 = const 1 and -h).
  D(wl)   [x,(k,h)] = x - h - cum via one contraction-3 matmul (PE).
  negA    = min(|D|,1)-1 = -tent(x-X): DVE copy + sign-bit clear for one
          384-chunk, Act Abs for the rest, one 4x DVE (min,sub).
  step2   po[co,h] += q[wl+k][:,k,:]^T @ negA[:,k,:]; lhsT=Q so output
          lands [co,h] pre-transposed; DVE drains to outC (GPSIMD cannot
          touch PSUM on real HW).
  GN      bn_stats -> group matmuls -> AllGather (15us vs AllReduce 28us
          in the cost model) -> local reduce -> relu(S*x+B) with the tent
          sign folded into S.
"""

import numpy as np
from contextlib import ExitStack

import concourse.bass as bass
import concourse.bacc as bacc
import concourse.tile as tile
import concourse.mybir as mybir
from concourse import bass_utils

F16 = mybir.dt.float16
F32 = mybir.dt.float32
U8 = mybir.dt.uint8
ALU = mybir.AluOpType
ACTF = mybir.ActivationFunctionType

K = 9
CENTER = 4
P = 128
W = 128
H = 128
B = 2
NY = 40            # sampling row window per core: y in [w0-4, w0+36)
WC = 32            # output w-rows per core
EPS = 1e-5
EXTEND = 1.0
BIG = 30000.0
NCORES = 8
NKH = K * P        # 1152
SPL = 864          # DVE share of the negA abs/min pass


def _tri_base():
    """tri[k, kp] = coeff of x_off[k] in cumulative offset new[kp]."""
    t = np.zeros((K, K), np.float32)
    t[0, 0] = 1.0
    t[K - 1, K - 1] = 1.0
    for i in range(1, CENTER):
        for j in range(CENTER + 1, CENTER + i + 1):
            t[j, CENTER + i] = 1.0
        for j in range(CENTER - i, CENTER):
            t[j, CENTER - i] = 1.0
    return t


def build_nc():
    nc = bacc.Bacc("TRN2", target_bir_lowering=False, debug=False,
                   num_devices=NCORES)

    fs_d = nc.dram_tensor("fs", [P, NY, P], F16, kind="ExternalInput")
    fcvh_d = nc.dram_tensor("fcvh", [P, 34, 130], F16, kind="ExternalInput")
    wt_d = nc.dram_tensor("wt", [P, NKH], F16, kind="ExternalInput")
    owh_d = nc.dram_tensor("owh", [P, 3, 96], F16, kind="ExternalInput")
    bvec_d = nc.dram_tensor("bvec", [K, 1], F32, kind="ExternalInput")
    triext_d = nc.dram_tensor("triext", [2 * K, K], F16, kind="ExternalInput")
    ybt_d = nc.dram_tensor("ybt", [K, WC * P], F16, kind="ExternalInput")
    hlo32_d = nc.dram_tensor("hlo32", [K, 3], F32, kind="ExternalInput")
    hhi32_d = nc.dram_tensor("hhi32", [K, 3], F32, kind="ExternalInput")
    big32_d = nc.dram_tensor("big32", [K, 1], F32, kind="ExternalInput")
    owl_d = nc.dram_tensor("owl", [P, 3, 96], F16, kind="ExternalInput")
    triext32_d = nc.dram_tensor("triext32", [2 * K, K], F32,
                                kind="ExternalInput")
    fcvbh_d = nc.dram_tensor("fcvbh", [P, 34, 10], F16, kind="ExternalInput")
    fcvbl_d = nc.dram_tensor("fcvbl", [P, 34, 10], F16, kind="ExternalInput")
    ybtb_d = nc.dram_tensor("ybtb", [K, WC * 6], F32, kind="ExternalInput")
    lhsT3_d = nc.dram_tensor("lhsT3", [3, P], F16, kind="ExternalInput")
    rhsC_d = nc.dram_tensor("rhsC", [2, WC * NKH], F16, kind="ExternalInput")
    gmat_d = nc.dram_tensor("gmat", [P, 32], F32, kind="ExternalInput")
    gexp_d = nc.dram_tensor("gexp", [32, P], F32, kind="ExternalInput")
    gam_d = nc.dram_tensor("gam", [P, 1], F32, kind="ExternalInput")
    bet_d = nc.dram_tensor("bet", [P, 1], F32, kind="ExternalInput")
    out_d = nc.dram_tensor("out", [P, WC, P], F32, kind="ExternalOutput")

    cr_in = nc.dram_tensor("cr_in", [32, 2], F32, kind="Internal")
    cr_out = nc.dram_tensor("cr_out", [4 * 32, 2], F32, kind="Internal")

    with tile.TileContext(nc) as tc, ExitStack() as ctx:
        const = ctx.enter_context(tc.tile_pool(name="const", bufs=1))
        mid = ctx.enter_context(tc.tile_pool(name="mid", bufs=1))

        def load(name, dram, shape, dtype):
            t = const.tile(shape, dtype, name=name + "_sb")
            nc.sync.dma_start(out=t, in_=dram.ap())
            return t

        owh = load("owh", owh_d, [P, 3, 96], F16)
        bvec = load("bvec", bvec_d, [K, 1], F32)
        triext = load("triext", triext_d, [2 * K, K], F16)
        fcvh = const.tile([P, 34, 130], F16, name="fcvh_sb")
        for lo, hi in ((0, 6), (6, 14), (14, 24), (24, 34)):
            nc.sync.dma_start(out=fcvh[:, lo:hi, :],
                              in_=fcvh_d.ap()[:, lo:hi, :])
        xoff18 = const.tile([2 * K, WC, P], F16, name="xoff18_sb")
        nc.sync.dma_start(
            out=xoff18.rearrange("p w h -> p (w h)")[K:2 * K, :],
            in_=ybt_d.ap())
        wt = load("wt", wt_d, [P, NKH], F16)
        fs = const.tile([P, NY, P], F16, name="fs_sb")
        for lo, hi in ((0, 10), (10, 24), (24, NY)):
            nc.sync.dma_start(out=fs[:, lo:hi, :],
                              in_=fs_d.ap()[:, lo:hi, :])
        hlo32 = load("hlo32", hlo32_d, [K, 3], F32)
        hhi32 = load("hhi32", hhi32_d, [K, 3], F32)
        big32 = load("big32", big32_d, [K, 1], F32)
        owl = load("owl", owl_d, [P, 3, 96], F16)
        triext32 = load("triext32", triext32_d, [2 * K, K], F32)
        fcvbh4 = load("fcvbh", fcvbh_d, [P, 34, 10], F16)
        fcvbl4 = load("fcvbl", fcvbl_d, [P, 34, 10], F16)
        fcvbh = fcvbh4.rearrange("p w (s c) -> p w s c", s=2)
        fcvbl = fcvbl4.rearrange("p w (s c) -> p w s c", s=2)
        xoffb32 = const.tile([2 * K, WC, 2, 3], F32, name="xoffb32_sb")
        nc.sync.dma_start(
            out=xoffb32.rearrange("p w s c -> p (w s c)")[K:2 * K, :],
            in_=ybtb_d.ap())
        lhsT3 = load("lhsT3", lhsT3_d, [3, P], F16)
        rhs3 = const.tile([3, WC * NKH], F16, name="rhs3_sb")
        nc.sync.dma_start(out=rhs3[0:2, :], in_=rhsC_d.ap())
        gmat = load("gmat", gmat_d, [P, 32], F32)
        gexp = load("gexp", gexp_d, [32, P], F32)
        gam = load("gam", gam_d, [P, 1], F32)
        bet = load("bet", bet_d, [P, 1], F32)

        braw16 = mid.tile([K, WC, P], F16)
        bb32 = mid.tile([K, WC, 2, 3], F32)
        outC = mid.tile([P, WC, P], F16)
        stats = mid.tile([P, 8, 6], F32)

        qtiles = {}
        qpool = ctx.enter_context(tc.tile_pool(name="qpool", bufs=13))
        napool = ctx.enter_context(tc.tile_pool(name="napool", bufs=5))
        fpool = ctx.enter_context(tc.tile_pool(name="fpool", bufs=4))
        psQ = ctx.enter_context(tc.tile_pool(name="psQ", bufs=5,
                                             space="PSUM"))
        psM = ctx.enter_context(tc.tile_pool(name="psM", bufs=1,
                                             space="PSUM"))

        nq = [0]

        def q_row(yl, engs="s"):
            q = qpool.tile([P, K, P], F16, tag="q", name="q_sb")
            qf = q.rearrange("p k c -> p (k c)")
            for j in range(3):
                psq = psQ.tile([P, 384], F32, tag="q", name="psq")
                nc.tensor.matmul(psq, lhsT=fs[:, yl, :],
                                 rhs=wt[:, j * 384:(j + 1) * 384],
                                 start=True, stop=True)
                dst = qf[:, j * 384:(j + 1) * 384]
                if engs[j % len(engs)] == "v":
                    nc.vector.tensor_copy(out=dst, in_=psq)
                else:
                    nc.scalar.copy(out=dst, in_=psq)
            nq[0] += 1
            qtiles[yl] = q

        # ---- offset branch (+ first 9 q rows interleaved) ----
        with tc.tile_pool(name="x9p", bufs=4) as x9p, \
             tc.tile_pool(name="psC", bufs=2, space="PSUM") as psC:
            wchunks = [(c3, min(3, WC - c3)) for c3 in range(0, WC, 3)]
            for ci_, (c3, nw) in enumerate(wchunks):
                pss = psC.tile([96, 3, 130], F32, tag="cv", name="pss")
                for dw in range(3):
                    nc.tensor.matmul(pss[:, :nw, :], lhsT=owh[:, dw, :],
                                     rhs=fcvh[:, c3 + dw: c3 + dw + nw, :],
                                     start=(dw == 0), stop=(dw == 2))
                g0 = x9p.tile([K, 3, P], F32, tag="g0", name="g0")
                g1 = x9p.tile([K, 3, P], F32, tag="g1", name="g1")
                x9t = x9p.tile([K, 3, P], F32, tag="x9", name="x9t")
                # the bir verifier allows only one PSUM input per op
                nc.scalar.copy(out=g0[:, :nw, :], in_=pss[0:K, :nw, 0:128])
                eng1 = nc.vector
                eng2 = nc.vector
                eng1.tensor_tensor(out=g1[:, :nw, :],
                                   in0=pss[32:32 + K, :nw, 1:129],
                                   in1=g0[:, :nw, :], op=ALU.add)
                eng2.tensor_tensor(out=x9t[:, :nw, :],
                                   in0=pss[64:64 + K, :nw, 2:130],
                                   in1=g1[:, :nw, :], op=ALU.add)
                nc.scalar.activation(out=xoff18[0:K, c3:c3 + nw, :],
                                     in_=x9t[:, :nw, :], func=ACTF.Tanh,
                                     bias=bvec, scale=1.0)
                pcum = psM.tile([K, 3, P], F32, tag="cm", name="pcum")
                nc.tensor.matmul(pcum[:, :nw, :], lhsT=triext,
                                 rhs=xoff18[:, c3:c3 + nw, :],
                                 start=True, stop=True)
                if ci_ % 2 == 0:
                    nc.vector.tensor_copy(out=braw16[:, c3:c3 + nw, 3:125],
                                          in_=pcum[:, :nw, 3:125])
                else:
                    nc.scalar.copy(out=braw16[:, c3:c3 + nw, 3:125],
                                   in_=pcum[:, :nw, 3:125])
                if ci_ < K:
                    q_row(ci_, engs=("vvs", "svv", "vsv")[ci_ % 3])

            # ---- precise boundary path ----
            # The bilinear clip makes the reference DISCONTINUOUS at X=0 and
            # X=127, which (|cum|<3) can only hit output columns h in
            # {0,1,2} u {125,126,127}. Those samples' cum must match the
            # fp32 reference to ~1e-5 or a sample can land on the wrong side
            # of the cliff (O(1) error). Recompute just the 10 affected fcv
            # columns with hi/lo f16 conv + fp32 cum, decide the masks there,
            # and overwrite braw16's 6 boundary columns.
            pssb = psC.tile([96, WC, 2, 5], F32, tag="cv", name="pssb")
            for dw in range(3):
                rh = fcvbh[:, dw:dw + WC, :, :]
                rl = fcvbl[:, dw:dw + WC, :, :]
                nc.tensor.matmul(pssb, lhsT=owh[:, dw, :], rhs=rh,
                                 start=(dw == 0), stop=False)
                nc.tensor.matmul(pssb, lhsT=owl[:, dw, :], rhs=rh,
                                 start=False, stop=False)
                nc.tensor.matmul(pssb, lhsT=owh[:, dw, :], rhs=rl,
                                 start=False, stop=(dw == 2))
            g0b = x9p.tile([K, WC, 2, 3], F32, tag="g0", name="g0b")
            g1b = x9p.tile([K, WC, 2, 3], F32, tag="g1", name="g1b")
            x9b = x9p.tile([K, WC, 2, 3], F32, tag="x9", name="x9b")
            nc.scalar.copy(out=g0b, in_=pssb[0:K, :, :, 0:3])
            nc.vector.tensor_tensor(out=g1b, in0=pssb[32:32 + K, :, :, 1:4],
                                    in1=g0b, op=ALU.add)
            nc.vector.tensor_tensor(out=x9b, in0=pssb[64:64 + K, :, :, 2:5],
                                    in1=g1b, op=ALU.add)
            nc.scalar.activation(out=xoffb32[0:K, :, :, :], in_=x9b,
                                 func=ACTF.Tanh, bias=bvec, scale=1.0)
            pcumb = psM.tile([K, WC, 2, 3], F32, tag="cm", name="pcumb")
            nc.tensor.matmul(pcumb, lhsT=triext32,
                             rhs=xoffb32.rearrange("p w s c -> p (w s c)"),
                             start=True, stop=True)
            bb32 = mid.tile([K, WC, 2, 3], F32)
            nc.vector.tensor_copy(out=bb32, in_=pcumb)
            m1 = mid.tile([K, WC, 3], U8)
            m2 = mid.tile([K, WC, 3], U8)
            nc.vector.tensor_tensor(
                out=m1, in0=bb32[:, :, 0, :],
                in1=hlo32.unsqueeze(1).to_broadcast([K, WC, 3]), op=ALU.is_gt)
            nc.vector.copy_predicated(
                out=bb32[:, :, 0, :], mask=m1,
                data=big32.to_broadcast([K, WC, 3]))
            nc.vector.tensor_tensor(
                out=m2, in0=bb32[:, :, 1, :],
                in1=hhi32.unsqueeze(1).to_broadcast([K, WC, 3]), op=ALU.is_le)
            nc.vector.copy_predicated(
                out=bb32[:, :, 1, :], mask=m2,
                data=big32.to_broadcast([K, WC, 3]))
            nc.vector.tensor_copy(out=braw16[:, :, 0:3], in_=bb32[:, :, 0, :])
            nc.vector.tensor_copy(out=braw16[:, :, 125:128],
                                  in_=bb32[:, :, 1, :])

        # fold braw16 into rhs3 row 2; row-2 layout is (k, wl, h) which is
        # exactly braw16's storage order. Two DMAs so D(0) gates only on
        # the first half.
        rhs3v = rhs3.rearrange("p (k w h) -> p k w h", k=K, h=P)
        nc.sync.dma_start(out=rhs3v[2:3, :, 0:16, :],
                          in_=braw16[:, 0:16, :])
        nc.sync.dma_start(out=rhs3v[2:3, :, 16:WC, :],
                          in_=braw16[:, 16:WC, :])

        # ---- main loop ----
        with tc.tile_pool(name="psD", bufs=2, space="PSUM") as psD:

            def emit_iter(wl):
                """Software-pipelined: build D/negA for wl+1, step2 for wl."""
                nxt = wl + 1
                na = None
                if nxt <= WC - 1:
                    na = napool.tile([P, K, P], F16, tag="na", name="na")
                    naf = na.rearrange("p k h -> p (k h)")
                    for j in range(3):
                        pd = psD.tile([P, 384], F32, tag="d", name="psd")
                        nc.tensor.matmul(
                            pd, lhsT=lhsT3,
                            rhs=rhs3v[:, 3 * j:3 * j + 3, nxt, :],
                            start=True, stop=True)
                        dst = naf[:, j * 384:(j + 1) * 384]
                        if j == 0:
                            nc.vector.tensor_copy(out=dst, in_=pd)
                        else:
                            nc.scalar.activation(out=dst, in_=pd,
                                                 func=ACTF.Abs)
                if 0 <= wl and wl + K < NY:
                    q_row(wl + K, engs="ssv")
                if na is not None:
                    # |d| for the DVE-copied part: clear the f16 sign bit
                    nau = naf[:, 0:384].bitcast(mybir.dt.uint16)
                    nc.vector.tensor_scalar(out=nau, in0=nau, scalar1=0x7FFF,
                                            scalar2=None, op0=ALU.bitwise_and)
                    nc.vector.tensor_scalar(out=naf, in0=naf, scalar1=1.0,
                                            scalar2=1.0, op0=ALU.min,
                                            op1=ALU.subtract)
                if wl < 0:
                    return na
                po = psM.tile([P, P], F32, tag="cm", name="po")
                na_c = nas.pop(wl)
                for k in range(K):
                    nc.tensor.matmul(po, lhsT=qtiles[wl + k][:, k, :],
                                     rhs=na_c[:, k, :],
                                     start=(k == 0), stop=(k == K - 1))
                nc.vector.tensor_copy(out=outC[:, wl, :], in_=po)
                if wl % 4 == 0 and wl > 0:
                    nc.vector.bn_stats(
                        out=stats[:, wl // 4 - 1, :],
                        in_=outC[:, wl - 4:wl, :].rearrange(
                            "p a b -> p (a b)"))
                del qtiles[wl]
                return na

            nas = {}
            nas[0] = emit_iter(-1)
            for wl in range(WC):
                na = emit_iter(wl)
                if na is not None:
                    nas[wl + 1] = na

            # ---- GroupNorm ----
            nc.vector.bn_stats(
                out=stats[:, 7, :],
                in_=outC[:, WC - 4:WC, :].rearrange("p a b -> p (a b)"))
            mv = mid.tile([P, 2], F32)
            nc.vector.bn_aggr(out=mv, in_=stats)
            st2 = mid.tile([P, 2], F32)
            nc.gpsimd.tensor_copy(out=st2[:, 0:1], in_=mv[:, 0:1])
            sq = mid.tile([P, 1], F32)
            nc.gpsimd.tensor_tensor(out=sq, in0=mv[:, 0:1], in1=mv[:, 0:1],
                                    op=ALU.mult)
            nc.gpsimd.tensor_tensor(out=st2[:, 1:2], in0=mv[:, 1:2], in1=sq,
                                    op=ALU.add)
            pg = psM.tile([32, 2], F32, tag="cm", name="ps_g")
            nc.tensor.matmul(pg, lhsT=gmat, rhs=st2, start=True, stop=True)
            g2 = mid.tile([32, 2], F32)
            nc.vector.tensor_copy(out=g2, in_=pg)
            nc.sync.dma_start(out=cr_in.ap(), in_=g2)
            nc.gpsimd.collective_compute(
                kind="AllGather", op=ALU.bypass,
                replica_groups=[[0, 1, 2, 3], [4, 5, 6, 7]],
                ins=[cr_in.ap()], outs=[cr_out.ap()])
            gg = mid.tile([32, 4, 2], F32)
            nc.sync.dma_start(
                out=gg, in_=cr_out.ap().rearrange("(r c) v -> c r v", r=4))
            g2s = mid.tile([32, 2], F32)
            # sum over the 4 cores: view [32, 2, 4] (v outer, r inner)
            nc.vector.tensor_reduce(
                out=g2s, in_=gg.rearrange("c r v -> c v r"),
                axis=mybir.AxisListType.X, op=ALU.add)
            m2 = mid.tile([32, 1], F32)
            nc.vector.tensor_tensor(out=m2, in0=g2s[:, 0:1], in1=g2s[:, 0:1],
                                    op=ALU.mult)
            vg = mid.tile([32, 1], F32)
            nc.vector.tensor_tensor(out=vg, in0=g2s[:, 1:2], in1=m2,
                                    op=ALU.subtract)
            nc.vector.tensor_scalar(out=vg, in0=vg, scalar1=EPS,
                                    scalar2=None, op0=ALU.add)
            nc.scalar.sqrt(out=vg, in_=vg)
            nc.vector.reciprocal(out=vg, in_=vg)
            g3 = mid.tile([32, 2], F32)
            nc.vector.tensor_copy(out=g3[:, 0:1], in_=g2s[:, 0:1])
            nc.vector.tensor_copy(out=g3[:, 1:2], in_=vg)
            pe2 = psM.tile([P, 2], F32, tag="cm", name="ps_e2")
            nc.tensor.matmul(pe2, lhsT=gexp, rhs=g3, start=True, stop=True)
            ec = mid.tile([P, 2], F32)
            nc.vector.tensor_copy(out=ec, in_=pe2)
            t1 = mid.tile([P, 1], F32)
            nc.vector.tensor_tensor(out=t1, in0=ec[:, 1:2], in1=gam,
                                    op=ALU.mult)
            Sv = mid.tile([P, 1], F32)
            nc.vector.tensor_scalar(out=Sv, in0=t1, scalar1=-1.0, scalar2=None,
                                    op0=ALU.mult)
            t2 = mid.tile([P, 1], F32)
            nc.vector.tensor_tensor(out=t2, in0=ec[:, 0:1], in1=t1,
                                    op=ALU.mult)
            Bv = mid.tile([P, 1], F32)
            nc.vector.tensor_tensor(out=Bv, in0=t2, in1=bet, op=ALU.add)
            blocks = ((0, 4), (4, 12), (12, 22), (22, WC))
            for b0, b1 in blocks:
                fin = fpool.tile([P, b1 - b0, P], F32, tag="f", name="fin")
                nc.scalar.activation(out=fin, in_=outC[:, b0:b1, :],
                                     func=ACTF.Relu, bias=Bv, scale=Sv)
                nc.sync.dma_start(out=out_d.ap()[:, b0:b1, :], in_=fin)

    nc.compile()
    return nc


_TRI = _tri_base()


def prep_shared(off_w, off_b, bn_gamma, bn_beta, bn_mean, bn_var, conv_w,
                gn_gamma, gn_beta):
    s36 = (np.asarray(bn_gamma, np.float32)
           / np.sqrt(np.asarray(bn_var, np.float32) + EPS))
    s = s36[K:2 * K]
    bvec = ((np.asarray(off_b, np.float32)[K:2 * K]
             - np.asarray(bn_mean, np.float32)[K:2 * K]) * s
            + np.asarray(bn_beta, np.float32)[K:2 * K]
            ).reshape(K, 1).astype(np.float32)

    owf = np.asarray(off_w, np.float32)[K:2 * K]          # [k, ci, dw, dh]
    oww = np.zeros((P, 3, 96), np.float32)                # [ci, dw, (dh-group, k)]
    for dw in range(3):
        for dh in range(3):
            oww[:, dw, dh * 32: dh * 32 + K] = (owf[:, :, dw, dh] * s[:, None]).T
    owh = oww.astype(np.float16)
    owl = (oww - owh.astype(np.float32)).astype(np.float16)

    wtf = np.asarray(conv_w, np.float32)[:, :, 0, :]      # [co, ci, k]
    wt = np.ascontiguousarray(
        np.transpose(wtf, (1, 2, 0)).reshape(P, K * P)).astype(np.float16)

    hx = np.arange(P, dtype=np.float32)
    mask7 = np.full((P, 1), 0x7FFF, np.uint16)
    triext32 = np.concatenate([-EXTEND * _TRI,
                               np.eye(K, dtype=np.float32)], axis=0)
    triext = triext32.astype(np.float16)
    lhsT3 = np.stack([hx, np.ones(P, np.float32),
                      np.ones(P, np.float32)]).astype(np.float16)
    rhsC = np.stack([np.ones(WC * K * P, np.float32),
                     np.tile(-hx, WC * K)]).astype(np.float16)
    return dict(
        wt=wt, owh=owh, owl=owl, bvec=bvec, triext=triext,
        triext32=triext32.astype(np.float32), lhsT3=lhsT3, rhsC=rhsC,
        hlo32=np.tile(np.arange(3, dtype=np.float32), (K, 1)),
        hhi32=np.tile(np.arange(3, dtype=np.float32) - 2.0, (K, 1)),
        big32=np.full((K, 1), BIG, np.float32),
        gam=np.asarray(gn_gamma, np.float32).reshape(P, 1),
        bet=np.asarray(gn_beta, np.float32).reshape(P, 1),
        gmat=np.array([[0.0625 if co // 4 == g else 0.0 for g in range(32)]
                       for co in range(P)], np.float32),
        gexp=np.array([[1.0 if co // 4 == g else 0.0 for co in range(P)]
                       for g in range(32)], np.float32),
    )


def prep_core(f, b, w0):
    fb = np.asarray(f, np.float32)[b]
    fs = np.zeros((P, NY, P), np.float16)
    lo = max(0, w0 - 4)
    hi = min(W, w0 + 36)
    fs[:, lo - (w0 - 4): hi - (w0 - 4), :] = fb[:, lo:hi, :].astype(np.float16)
    fpad = np.pad(fb, ((0, 0), (1, 1), (1, 1)))
    fcvh = np.ascontiguousarray(fpad[:, w0:w0 + 34, :]).astype(np.float16)
    b10 = [0, 1, 2, 3, 4, 125, 126, 127, 128, 129]
    fcvb = np.ascontiguousarray(fpad[:, w0:w0 + 34, :][:, :, b10])
    fcvbh = fcvb.astype(np.float16)
    fcvbl = (fcvb - fcvbh.astype(np.float32)).astype(np.float16)
    ybt = np.zeros((K, WC, P), np.float16)
    ybtb = np.zeros((K, WC, 6), np.float32)
    for wl in range(WC):
        for k in range(K):
            y = w0 + wl + k - 4
            if not (0 <= y <= 126):
                ybt[k, wl, :] = BIG
                ybtb[k, wl, :] = BIG
    return dict(fs=fs, fcvh=fcvh, ybt=ybt.reshape(K, WC * P),
                fcvbh=fcvbh, fcvbl=fcvbl, ybtb=ybtb.reshape(K, WC * 6))


_NC_CACHE = {}


def get_nc():
    if "nc" not in _NC_CACHE:
        _NC_CACHE["nc"] = build_nc()
    return _NC_CACHE["nc"]


def make_in_maps(f, off_w, off_b, bn_gamma, bn_beta, bn_mean, bn_var,
                 conv_w, conv_b, gn_gamma, gn_beta):
    consts = prep_shared(off_w, off_b, bn_gamma, bn_beta, bn_mean, bn_var,
                         conv_w, gn_gamma, gn_beta)
    in_maps = []
    for c in range(NCORES):
        b, q = c // 4, c % 4
        m = dict(consts)
        m.update(prep_core(f, b, q * WC))
        in_maps.append(m)
    return in_maps


def assemble(results):
    out = np.zeros((B, P, W, H), np.float32)
    for c in range(NCORES):
        b, q = c // 4, c % 4
        out[b, :, q * WC:(q + 1) * WC, :] = results[c]["out"]
    return out


def kernel(f, off_w, off_b, bn_gamma, bn_beta, bn_mean, bn_var,
           conv_w, conv_b, gn_gamma, gn_beta, **run_kwargs):
    nc = get_nc()
    in_maps = make_in_maps(f, off_w, off_b, bn_gamma, bn_beta, bn_mean,
                           bn_var, conv_w, conv_b, gn_gamma, gn_beta)
    last_exc = None
    for _attempt in range(3):
        try:
            res = bass_utils.run_bass_kernel_spmd(
                nc, in_maps, core_ids=list(range(NCORES)), **run_kwargs)
            break
        except Exception as e:  # transient tunnel/device hiccups
            last_exc = e
    else:
        raise last_exc
    out = assemble(res.results)
    kernel.last_result = res
    return out
